# revision 5
# baseline (speedup 1.0000x reference)
"""Trainium2 Bass kernel for the AdapterController hard-routing MoE adapter.

Reference computation (per router m in [0,4), batch b in [0,16)):
    e = expert_index[m, b]
    z = x[b] @ down_w[m, e] + down_b[m, e]      # [512, 256]
    z = z * sigmoid(z)                          # swish
    u = z @ up_w[m, e]                          # [512, 1024]
    out[m, b] = u

Strategy: data-parallel over the batch axis (2 batches per core, 8 cores).
The expert gather is part of input sharding: each core receives exactly the
(m, b)-selected weight matrices, packed on the host into the SBUF partition
layout so every DMA is fully contiguous.

On-chip per (m, b) pair:
    zT[d, s] = sum_c Wd[c, d] * xT[c, s]        (16 matmuls N=512, K=128)
    z = silu(zT + bd)                           (ACT engine, PSUM -> SBUF)
    u[s, c] = sum_d zT[d, s].T @ Wu[d, c]       (16 matmuls N=512)

MODE selects compute/transfer dtypes:
    "f32r":  f32 DMA, float32r matmuls (TF32-like, full PE rate)
    "bf16":  bf16 weights/x (host-cast), f32 output
    "bf16o": bf16 weights/x and bf16 output (host-upcast to f32)
"""

import numpy as np

MODE = "bf16o"

M, B, S, C, D = 4, 16, 512, 1024, 256
N_CORES = 8
B_LOC = B // N_CORES  # batches per core
KC = C // 128         # 8 c-chunks
KD = D // 128         # 2 d-chunks
KS = S // 128         # 4 s-chunks

_cache = {}
last_results = None  # BassKernelResults of the most recent run (for test.py)


def _build(mode):
    from contextlib import ExitStack

    import concourse.mybir as mybir
    import concourse.tile as tile
    from concourse import bacc

    f32 = mybir.dt.float32
    bf16 = mybir.dt.bfloat16
    in_dt = f32 if mode == "f32r" else bf16
    mm_dt = mybir.dt.float32r if mode == "f32r" else bf16
    out_dt = bf16 if mode == "bf16o" else f32

    nc = bacc.Bacc("TRN2", target_bir_lowering=False, debug=False,
                   num_devices=N_CORES)
    # xtp[b, half][p, k*512 + s] = x[b, s, 128*(4*half + k) + p]
    xtp = nc.dram_tensor("xtp", [B_LOC, 2, 128, KC * S // 2], in_dt,
                         kind="ExternalInput").ap()
    # wdp[m, b][p, k*256 + d] = down_w_gathered[m, b, 128k + p, d]
    wdp = nc.dram_tensor("wdp", [M, B_LOC, 128, KC * D], in_dt,
                         kind="ExternalInput").ap()
    # wup[m, b][p, j*1024 + c] = up_w_gathered[m, b, 128j + p, c]
    wup = nc.dram_tensor("wup", [M, B_LOC, 128, KD * C], in_dt,
                         kind="ExternalInput").ap()
    # bdp[p, (m*B_LOC+b)*2 + j] = down_b_gathered[m, b, 128j + p]
    bdp = nc.dram_tensor("bdp", [128, M * B_LOC * KD], f32,
                         kind="ExternalInput").ap()
    out = nc.dram_tensor("out", [M, B_LOC, S, C], out_dt,
                         kind="ExternalOutput").ap()

    silu = mybir.ActivationFunctionType.Silu
    copy_fn = mybir.ActivationFunctionType.Copy

    def load(engine, dst, src):
        # casting DMA must go through SWDGE (gpsimd); plain DMA via HWDGE
        if mode == "f32r":
            nc.gpsimd.dma_start(dst, src)
        else:
            engine.dma_start(dst, src)

    with tile.TileContext(nc) as tc, ExitStack() as ctx:
        const = ctx.enter_context(tc.tile_pool(name="const", bufs=1))
        xpool = ctx.enter_context(tc.tile_pool(name="xpool", bufs=4))
        wdpool = ctx.enter_context(tc.tile_pool(name="wdpool", bufs=3))
        wupool = ctx.enter_context(tc.tile_pool(name="wupool", bufs=3))
        zpool = ctx.enter_context(tc.tile_pool(name="zpool", bufs=2))
        upool = ctx.enter_context(tc.tile_pool(name="upool", bufs=3))
        pszp = ctx.enter_context(tc.tile_pool(name="pszp", bufs=2, space="PSUM"))
        psup = ctx.enter_context(tc.tile_pool(name="psup", bufs=3, space="PSUM"))

        bd_sb = const.tile([128, M * B_LOC * KD], f32)
        nc.scalar.dma_start(bd_sb[:], bdp[:])

        for b in range(B_LOC):
            # x transposed, in two half-loads so compute starts earlier
            xh = []
            for half in range(2):
                xt_sb = xpool.tile([128, KC * S // 2], mm_dt, tag="xt")
                load(nc.sync, xt_sb[:], xtp[b, half])
                xh.append(xt_sb)

            for m in range(M):
                wd_sb = wdpool.tile([128, KC * D], mm_dt)
                load(nc.scalar, wd_sb[:], wdp[m, b])
                wu_sb = wupool.tile([128, KD * C], mm_dt)
                load(nc.scalar, wu_sb[:], wup[m, b])

                z_sb = zpool.tile([128, KD, S], mm_dt)
                for j in range(KD):
                    psz = pszp.tile([128, S], f32)
                    for k in range(KC):
                        nc.tensor.matmul(
                            psz[:],
                            wd_sb[:, k * 256 + j * 128: k * 256 + j * 128 + 128],
                            xh[k // 4][:, (k % 4) * S: (k % 4 + 1) * S],
                            start=(k == 0), stop=(k == KC - 1),
                        )
                    col = (m * B_LOC + b) * KD + j
                    nc.scalar.activation(z_sb[:, j, :], psz[:], silu,
                                         bias=bd_sb[:, col: col + 1])

                for a in range(KS):
                    # one 2-bank PSUM tile holds the full [128, 1024] u row;
                    # j-outer order lets j=0 matmuls run before silu(j=1) lands
                    psu = psup.tile([128, C], f32)
                    for j in range(KD):
                        for h in range(2):
                            nc.tensor.matmul(
                                psu[:, h * 512: (h + 1) * 512],
                                z_sb[:, j, a * 128: (a + 1) * 128],
                                wu_sb[:, j * 1024 + h * 512:
                                      j * 1024 + h * 512 + 512],
                                start=(j == 0), stop=(j == KD - 1),
                                skip_group_check=True,
                            )
                    u_sb = upool.tile([128, C], out_dt, tag="u")
                    if a % 2 == 0:
                        nc.vector.tensor_copy(u_sb[:], psu[:])
                    else:
                        nc.scalar.activation(u_sb[:], psu[:], copy_fn)
                    nc.sync.dma_start(
                        out[m, b, a * 128:(a + 1) * 128, :], u_sb[:])

    nc.compile()
    return nc


def _get_nc(mode):
    if mode not in _cache:
        _cache[mode] = _build(mode)
    return _cache[mode]


def kernel(x, expert_index, down_w, down_b, up_w):
    global last_results
    import ml_dtypes
    from concourse import bass_utils

    x = np.asarray(x, dtype=np.float32)
    idx = np.asarray(expert_index)
    r = np.arange(M)[:, None]
    wd = np.asarray(down_w, dtype=np.float32)[r, idx]   # [M, B, C, D]
    bd = np.asarray(down_b, dtype=np.float32)[r, idx]   # [M, B, D]
    wu = np.asarray(up_w, dtype=np.float32)[r, idx]     # [M, B, D, C]

    # Pack into SBUF partition-major layouts (see _build comments).
    xt = x.transpose(0, 2, 1).reshape(B, 2, KC // 2, 128, S)
    xt = xt.transpose(0, 1, 3, 2, 4).reshape(B, 2, 128, KC * S // 2)
    wdp = wd.reshape(M, B, KC, 128, D).transpose(0, 1, 3, 2, 4)
    wdp = wdp.reshape(M, B, 128, KC * D)
    wup = wu.reshape(M, B, KD, 128, C).transpose(0, 1, 3, 2, 4)
    wup = wup.reshape(M, B, 128, KD * C)
    bdp = bd.reshape(M, B, KD, 128).transpose(3, 0, 1, 2)  # [128, M, B, KD]

    in_dt = np.float32 if MODE == "f32r" else ml_dtypes.bfloat16

    in_maps = []
    for i in range(N_CORES):
        bs = slice(i * B_LOC, (i + 1) * B_LOC)
        cols = bdp[:, :, bs, :].reshape(128, M * B_LOC * KD)
        in_maps.append({
            "xtp": np.ascontiguousarray(xt[bs].astype(in_dt)),
            "wdp": np.ascontiguousarray(wdp[:, bs].astype(in_dt)),
            "wup": np.ascontiguousarray(wup[:, bs].astype(in_dt)),
            "bdp": np.ascontiguousarray(cols.astype(np.float32)),
        })

    nc = _get_nc(MODE)
    res = bass_utils.run_bass_kernel_spmd(nc, in_maps,
                                          core_ids=list(range(N_CORES)))
    last_results = res

    full = np.empty((M, B, S, C), dtype=np.float32)
    for i in range(N_CORES):
        full[:, i * B_LOC:(i + 1) * B_LOC] = np.asarray(
            res.results[i]["out"]).astype(np.float32)
    return full
